# revision 2
# baseline (speedup 1.0000x reference)
"""Multi-head attention TRN2 Bass kernel.

Problem: B=2, S=2048, H=1024, N=16 heads, D=64.
  q,k,v = einsum('bsh,nhd->bnsd', x, W*)
  scores = q @ k^T / 8 ; attn = softmax(scores) ; ctx = attn @ v
  out = concat(ctx) @ Wo ; returns (out, attn)

Sharding over 8 cores: core c -> batch b = c//4, head group g = c%4
(4 heads per core).  Each core computes its 4 heads for its batch:
attn_part [4,2048,2048] plus a partial output projection
out_part [2048,1024] (contraction over its 256 concat channels).
Host sums the 4 partials per batch (the "all-reduce") and stacks attn.

Core-local pipeline (all matmuls in float32r ~ 1.2e-4 rel precision):
  P) X^T via PE transposes; qT/kT [c=2x128, S] and v [S, 4*64] projections.
  A) per (head, 128-row tile): scores (K=64 matmuls) -> PSUM; exp via ACT
     (scale=1/8, accum rowsums); reciprocal; normalize (GPSIMD); DMA attn
     out; PE-transpose attn tiles; evac to f32r; ctx matmul per 256-col
     chunk accumulating over k.
  O) out_part = ctxT.T @ Wo_rows via 2-chunk contraction; DMA out.
"""

import numpy as np

import concourse.bass as bass
import concourse.mybir as mybir
import concourse.tile as tile
from concourse import bacc, bass_utils
from concourse.masks import make_identity

F32 = mybir.dt.float32
F32R = mybir.dt.float32r
EXP = mybir.ActivationFunctionType.Exp

B, S, H, N, D = 2, 2048, 1024, 16, 64
NH = 4          # heads per core
NC = 8          # cores
JT = S // 128   # 16 row tiles
HC = H // 128   # 8 contraction chunks


def build_program():
    nc = bacc.Bacc("TRN2", target_bir_lowering=False, debug=False)

    xq = nc.dram_tensor("xq", [S, H], F32R, kind="ExternalInput")
    xk = nc.dram_tensor("xk", [S, H], F32R, kind="ExternalInput")
    xv = nc.dram_tensor("xv", [S, H], F32R, kind="ExternalInput")
    wq = nc.dram_tensor("wq", [H, NH * D], F32R, kind="ExternalInput")
    wk = nc.dram_tensor("wk", [H, NH * D], F32R, kind="ExternalInput")
    wv = nc.dram_tensor("wv", [H, NH * D], F32R, kind="ExternalInput")
    wo = nc.dram_tensor("wo", [NH * D, H], F32R, kind="ExternalInput")
    attn_o = nc.dram_tensor("attn_o", [NH, S, S], F32, kind="ExternalOutput")
    out_o = nc.dram_tensor("out_o", [S, H], F32, kind="ExternalOutput")

    with tile.TileContext(nc) as tc:
        with (
            tc.tile_pool(name="persist", bufs=1) as perst,
        ):
            id32 = perst.tile([128, 128], F32)
            make_identity(nc, id32[:])
            idr = perst.tile([128, 128], F32R)
            nc.vector.tensor_copy(idr[:], id32[:])

            wo_sb = perst.tile([128, 2, H], F32R)
            nc.sync.dma_start(wo_sb[:], wo.rearrange("(c p) n -> p c n", p=128))

            # qT / kT: [channel-in-pair 128, pair 2, S]; pair hp holds heads
            # 2hp, 2hp+1 (channel = (h%2)*64 + d)
            qt_sb = perst.tile([128, 2, S], F32R)
            kt_sb = perst.tile([128, 2, S], F32R)
            # v: [row-in-tile 128, tile 16, head*64+d 256]
            v_sb = perst.tile([128, JT, NH * D], F32R)
            # ctx^T: [channel%128, chunk 2, S] (channel = head*64+d)
            ctxT_sb = perst.tile([128, 2, S], F32R)

            # ---------------- Phase P: projections ----------------
            with (
                tc.tile_pool(name="wpool", bufs=1) as wpool,
                tc.tile_pool(name="xpool", bufs=2) as xpool,
                tc.tile_pool(name="xtpool", bufs=2) as xtpool,
                tc.tile_pool(name="psumP", bufs=2, space="PSUM") as psumP,
            ):
                w_sbs = {}
                for name, wd in (("q", wq), ("k", wk), ("v", wv)):
                    w_sbs[name] = wpool.tile([128, HC, NH * D], F32R,
                                             tag=f"w{name}", name=f"w{name}_sb")
                    nc.sync.dma_start(
                        w_sbs[name][:], wd.rearrange("(c p) n -> p c n", p=128))

                for name, xd in (("q", xq), ("k", xk), ("v", xv)):
                    for jc in range(4):  # chunks of 512 rows
                        x_t = xpool.tile([128, 4, H], F32R, tag="x")
                        nc.sync.dma_start(
                            x_t[:],
                            xd[jc * 512:(jc + 1) * 512, :]
                            .rearrange("(a p) h -> p a h", p=128))
                        xt_t = xtpool.tile([128, HC, 512], F32R, tag="xt")
                        for hc in range(HC):
                            ps = psumP.tile([128, 4, 128], F32R, tag="ptr")
                            for jt in range(4):
                                nc.tensor.transpose(
                                    ps[:, jt],
                                    x_t[:, jt, hc * 128:(hc + 1) * 128],
                                    idr[:])
                            nc.vector.tensor_copy(xt_t[:, hc], ps[:])
                        if name in ("q", "k"):
                            dest = qt_sb if name == "q" else kt_sb
                            for hp in range(2):
                                pq = psumP.tile([128, 512], F32, tag="pqk")
                                for hc in range(HC):
                                    nc.tensor.matmul(
                                        pq[:],
                                        w_sbs[name][:, hc,
                                                    hp * 128:(hp + 1) * 128],
                                        xt_t[:, hc],
                                        start=(hc == 0), stop=(hc == HC - 1))
                                nc.scalar.copy(
                                    dest[:, hp, jc * 512:(jc + 1) * 512],
                                    pq[:])
                        else:
                            for jt in range(4):
                                pv = psumP.tile([128, NH * D], F32, tag="pv")
                                for hc in range(HC):
                                    nc.tensor.matmul(
                                        pv[:],
                                        xt_t[:, hc, jt * 128:(jt + 1) * 128],
                                        w_sbs["v"][:, hc],
                                        start=(hc == 0), stop=(hc == HC - 1))
                                nc.vector.tensor_copy(
                                    v_sb[:, jc * 4 + jt], pv[:])

            # ---------------- Phase A: attention ----------------
            with (
                tc.tile_pool(name="epool", bufs=3) as epool,
                tc.tile_pool(name="atpool", bufs=2) as atpool,
                tc.tile_pool(name="spool", bufs=4) as spool,
                tc.tile_pool(name="psumS", bufs=2, space="PSUM") as psumS,
                tc.tile_pool(name="psumT", bufs=2, space="PSUM") as psumT,
                tc.tile_pool(name="psumC", bufs=2, space="PSUM") as psumC,
            ):
                for h in range(NH):
                    hp, ho = h // 2, (h % 2) * 64
                    for qc in range(JT // 2):  # 256-row ctx chunks
                        at_t = atpool.tile([128, JT, 256], F32R, tag="at")
                        for mi in range(2):
                            m = qc * 2 + mi
                            # scores for row tile m: two [128,1024] psums
                            exp_t = epool.tile([128, S], F32, tag="exp")
                            sums2 = spool.tile([128, 2], F32, tag="sums")
                            ssum = spool.tile([128, 1], F32, tag="ssum")
                            rcp = spool.tile([128, 1], F32, tag="rcp")
                            for half in range(2):
                                ps_s = psumS.tile([128, 2, 512], F32,
                                                  tag="ps")
                                for n in range(2):
                                    kc = half * 2 + n
                                    nc.tensor.matmul(
                                        ps_s[:, n],
                                        qt_sb[ho:ho + 64, hp,
                                              m * 128:(m + 1) * 128],
                                        kt_sb[ho:ho + 64, hp,
                                              kc * 512:(kc + 1) * 512],
                                        start=True, stop=True)
                                nc.scalar.activation(
                                    exp_t[:, half * 1024:(half + 1) * 1024],
                                    ps_s[:], EXP, scale=0.125,
                                    accum_out=sums2[:, half:half + 1])
                            nc.vector.tensor_add(
                                ssum[:], sums2[:, 0:1], sums2[:, 1:2])
                            nc.vector.reciprocal(rcp[:], ssum[:])
                            nc.gpsimd.tensor_scalar_mul(
                                exp_t[:], exp_t[:], rcp[:])
                            nc.sync.dma_start(
                                attn_o[h, m * 128:(m + 1) * 128, :], exp_t[:])
                            # transpose the normalized tile for ctx
                            for g in range(4):
                                ps_t = psumT.tile([128, 4, 128], F32,
                                                  tag="pt")
                                for t4 in range(4):
                                    nc.tensor.transpose(
                                        ps_t[:, t4],
                                        exp_t[:, (g * 4 + t4) * 128:
                                              (g * 4 + t4 + 1) * 128],
                                        id32[:])
                                dst = at_t[:, g * 4:(g + 1) * 4,
                                           mi * 128:(mi + 1) * 128]
                                if g % 2 == 0:
                                    nc.vector.tensor_copy(dst, ps_t[:])
                                else:
                                    nc.scalar.copy(dst, ps_t[:])
                        # ctx for this 256-col chunk
                        pc = psumC.tile([64, 256], F32, tag="pc")
                        for kt in range(JT):
                            nc.tensor.matmul(
                                pc[:], v_sb[:, kt, h * 64:(h + 1) * 64],
                                at_t[:, kt], start=(kt == 0),
                                stop=(kt == JT - 1))
                        nc.vector.tensor_copy(
                            ctxT_sb[ho:ho + 64, hp,
                                    qc * 256:(qc + 1) * 256], pc[:])

            # ---------------- Phase O: output projection ----------------
            with (
                tc.tile_pool(name="opool", bufs=2) as opool,
                tc.tile_pool(name="psumO", bufs=4, space="PSUM") as psumO,
            ):
                for mt in range(JT):
                    out_t = opool.tile([128, H], F32, tag="out")
                    for n in range(2):
                        po = psumO.tile([128, 512], F32, tag="po")
                        for cc in range(2):
                            nc.tensor.matmul(
                                po[:],
                                ctxT_sb[:, cc, mt * 128:(mt + 1) * 128],
                                wo_sb[:, cc, n * 512:(n + 1) * 512],
                                start=(cc == 0), stop=(cc == 1))
                        nc.scalar.copy(out_t[:, n * 512:(n + 1) * 512], po[:])
                    nc.sync.dma_start(
                        out_o[mt * 128:(mt + 1) * 128, :], out_t[:])

    nc.compile()
    return nc


_NC_CACHE = None


def _get_program():
    global _NC_CACHE
    if _NC_CACHE is None:
        _NC_CACHE = build_program()
    return _NC_CACHE


def _make_in_maps(query, key, value, Wq, Wk, Wv, Wo):
    query = np.asarray(query, dtype=np.float32)
    key = np.asarray(key, dtype=np.float32)
    value = np.asarray(value, dtype=np.float32)
    Wq = np.asarray(Wq, dtype=np.float32)
    Wk = np.asarray(Wk, dtype=np.float32)
    Wv = np.asarray(Wv, dtype=np.float32)
    Wo = np.asarray(Wo, dtype=np.float32)

    # [N,H,D] -> [H, N*D]
    wq_f = np.ascontiguousarray(Wq.transpose(1, 0, 2).reshape(H, N * D))
    wk_f = np.ascontiguousarray(Wk.transpose(1, 0, 2).reshape(H, N * D))
    wv_f = np.ascontiguousarray(Wv.transpose(1, 0, 2).reshape(H, N * D))

    in_maps = []
    for c in range(NC):
        b, g = c // 4, c % 4
        cols = slice(g * NH * D, (g + 1) * NH * D)
        in_maps.append({
            "xq": np.ascontiguousarray(query[b]),
            "xk": np.ascontiguousarray(key[b]),
            "xv": np.ascontiguousarray(value[b]),
            "wq": np.ascontiguousarray(wq_f[:, cols]),
            "wk": np.ascontiguousarray(wk_f[:, cols]),
            "wv": np.ascontiguousarray(wv_f[:, cols]),
            "wo": np.ascontiguousarray(Wo[cols, :]),
        })
    return in_maps


def run(trace=False, **inputs):
    nc = _get_program()
    in_maps = _make_in_maps(**inputs)
    res = bass_utils.run_bass_kernel_spmd(
        nc, in_maps, core_ids=list(range(NC)), trace=trace)
    out = np.zeros((B, S, H), dtype=np.float32)
    attn = np.empty((B, N, S, S), dtype=np.float32)
    for c in range(NC):
        b, g = c // 4, c % 4
        r = res.results[c]
        attn[b, g * NH:(g + 1) * NH] = r["attn_o"]
        out[b] += r["out_o"]
    return (out, attn), res


def kernel(**inputs):
    (out, attn), _ = run(trace=False, **inputs)
    return out, attn


# revision 3
# speedup vs baseline: 4.6677x; 4.6677x over previous
"""Multi-head attention TRN2 Bass kernel.

Problem: B=2, S=2048, H=1024, N=16 heads, D=64.
  q,k,v = einsum('bsh,nhd->bnsd', x, W*)
  scores = q @ k^T / 8 ; attn = softmax(scores) ; ctx = attn @ v
  out = concat(ctx) @ Wo ; returns (out, attn)

Sharding over 8 cores: core c -> batch b = c//4, head group g = c%4
(4 heads per core).  Each core computes its 4 heads for its batch:
attn_part [4,2048,2048] plus a partial output projection
out_part [2048,1024] (contraction over its 256 concat channels).
Host sums the 4 partials per batch (the "all-reduce") and stacks attn.

Core-local pipeline (all matmuls in float32r ~ 1.2e-4 rel precision):
  P) X^T via PE transposes; qT/kT [c=2x128, S] and v [S, 4*64] projections.
  A) per (head, 128-row tile): scores (K=64 matmuls) -> PSUM; exp via ACT
     (scale=1/8, accum rowsums); reciprocal; normalize (GPSIMD); DMA attn
     out; PE-transpose attn tiles; evac to f32r; ctx matmul per 256-col
     chunk accumulating over k.
  O) out_part = ctxT.T @ Wo_rows via 2-chunk contraction; DMA out.
"""

import numpy as np

import concourse.bass as bass
import concourse.mybir as mybir
import concourse.tile as tile
from concourse import bacc, bass_utils
from concourse.masks import make_identity

F32 = mybir.dt.float32
F32R = mybir.dt.float32r
EXP = mybir.ActivationFunctionType.Exp

B, S, H, N, D = 2, 2048, 1024, 16, 64
NH = 4          # heads per core
NC = 8          # cores
JT = S // 128   # 16 row tiles
HC = H // 128   # 8 contraction chunks


def build_program():
    nc = bacc.Bacc("TRN2", target_bir_lowering=False, debug=False)

    xq = nc.dram_tensor("xq", [S, H], F32R, kind="ExternalInput")
    xk = nc.dram_tensor("xk", [S, H], F32R, kind="ExternalInput")
    xv = nc.dram_tensor("xv", [S, H], F32R, kind="ExternalInput")
    wq = nc.dram_tensor("wq", [H, NH * D], F32R, kind="ExternalInput")
    wk = nc.dram_tensor("wk", [H, NH * D], F32R, kind="ExternalInput")
    wv = nc.dram_tensor("wv", [H, NH * D], F32R, kind="ExternalInput")
    wo = nc.dram_tensor("wo", [NH * D, H], F32R, kind="ExternalInput")
    attn_o = nc.dram_tensor("attn_o", [NH, S, S], F32, kind="ExternalOutput")
    out_o = nc.dram_tensor("out_o", [S, H], F32, kind="ExternalOutput")

    with tile.TileContext(nc) as tc:
        with (
            tc.tile_pool(name="persist", bufs=1) as perst,
        ):
            id32 = perst.tile([128, 128], F32)
            make_identity(nc, id32[:])
            idr = perst.tile([128, 128], F32R)
            nc.vector.tensor_copy(idr[:], id32[:])

            wo_sb = perst.tile([128, 2, H], F32R)
            nc.sync.dma_start(wo_sb[:], wo.rearrange("(c p) n -> p c n", p=128))

            # qT / kT: [channel-in-pair 128, pair 2, S]; pair hp holds heads
            # 2hp, 2hp+1 (channel = (h%2)*64 + d)
            qt_sb = perst.tile([128, 2, S], F32R)
            kt_sb = perst.tile([128, 2, S], F32R)
            # v: [row-in-tile 128, tile 16, head*64+d 256]
            v_sb = perst.tile([128, JT, NH * D], F32R)
            # ctx^T: [channel%128, chunk 2, S] (channel = head*64+d)
            ctxT_sb = perst.tile([128, 2, S], F32R)

            # ---------------- Phase P: projections ----------------
            with (
                tc.tile_pool(name="wpool", bufs=1) as wpool,
                tc.tile_pool(name="xpool", bufs=2) as xpool,
                tc.tile_pool(name="xtpool", bufs=2) as xtpool,
                tc.tile_pool(name="psumP", bufs=2, space="PSUM") as psumP,
            ):
                w_sbs = {}
                for name, wd in (("q", wq), ("k", wk), ("v", wv)):
                    w_sbs[name] = wpool.tile([128, HC, NH * D], F32R,
                                             tag=f"w{name}", name=f"w{name}_sb")
                    nc.sync.dma_start(
                        w_sbs[name][:], wd.rearrange("(c p) n -> p c n", p=128))

                for name, xd in (("q", xq), ("k", xk), ("v", xv)):
                    for jc in range(4):  # chunks of 512 rows
                        x_t = xpool.tile([128, 4, H], F32R, tag="x")
                        nc.sync.dma_start(
                            x_t[:],
                            xd[jc * 512:(jc + 1) * 512, :]
                            .rearrange("(a p) h -> p a h", p=128))
                        xt_t = xtpool.tile([128, HC, 512], F32R, tag="xt")
                        for hc in range(HC):
                            ps = psumP.tile([128, 4, 128], F32R, tag="ptr")
                            for jt in range(4):
                                nc.tensor.transpose(
                                    ps[:, jt],
                                    x_t[:, jt, hc * 128:(hc + 1) * 128],
                                    idr[:])
                            nc.vector.tensor_copy(xt_t[:, hc], ps[:])
                        if name in ("q", "k"):
                            dest = qt_sb if name == "q" else kt_sb
                            for hp in range(2):
                                pq = psumP.tile([128, 512], F32, tag="pqk")
                                for hc in range(HC):
                                    nc.tensor.matmul(
                                        pq[:],
                                        w_sbs[name][:, hc,
                                                    hp * 128:(hp + 1) * 128],
                                        xt_t[:, hc],
                                        start=(hc == 0), stop=(hc == HC - 1))
                                nc.scalar.copy(
                                    dest[:, hp, jc * 512:(jc + 1) * 512],
                                    pq[:])
                        else:
                            for jt in range(4):
                                pv = psumP.tile([128, NH * D], F32, tag="pv")
                                for hc in range(HC):
                                    nc.tensor.matmul(
                                        pv[:],
                                        xt_t[:, hc, jt * 128:(jt + 1) * 128],
                                        w_sbs["v"][:, hc],
                                        start=(hc == 0), stop=(hc == HC - 1))
                                nc.vector.tensor_copy(
                                    v_sb[:, jc * 4 + jt], pv[:])

            # ---------------- Phase A: attention ----------------
            with (
                tc.tile_pool(name="epool", bufs=3) as epool,
                tc.tile_pool(name="atpool", bufs=2) as atpool,
                tc.tile_pool(name="spool", bufs=4) as spool,
                tc.tile_pool(name="psumS", bufs=2, space="PSUM") as psumS,
                tc.tile_pool(name="psumT", bufs=2, space="PSUM") as psumT,
                tc.tile_pool(name="psumC", bufs=2, space="PSUM") as psumC,
            ):
                for h in range(NH):
                    hp, ho = h // 2, (h % 2) * 64
                    for qc in range(JT // 2):  # 256-row ctx chunks
                        at_t = atpool.tile([128, JT, 256], F32R, tag="at")
                        for mi in range(2):
                            m = qc * 2 + mi
                            # scores for row tile m: two [128,1024] psums
                            exp_t = epool.tile([128, S], F32, tag="exp")
                            sums2 = spool.tile([128, 2], F32, tag="sums")
                            ssum = spool.tile([128, 1], F32, tag="ssum")
                            rcp = spool.tile([128, 1], F32, tag="rcp")
                            for half in range(2):
                                ps_s = psumS.tile([128, 2, 512], F32,
                                                  tag="ps")
                                for n in range(2):
                                    kc = half * 2 + n
                                    nc.tensor.matmul(
                                        ps_s[:, n],
                                        qt_sb[ho:ho + 64, hp,
                                              m * 128:(m + 1) * 128],
                                        kt_sb[ho:ho + 64, hp,
                                              kc * 512:(kc + 1) * 512],
                                        start=True, stop=True)
                                nc.scalar.activation(
                                    exp_t[:, half * 1024:(half + 1) * 1024],
                                    ps_s[:], EXP, scale=0.125,
                                    accum_out=sums2[:, half:half + 1])
                            nc.vector.tensor_add(
                                ssum[:], sums2[:, 0:1], sums2[:, 1:2])
                            nc.vector.reciprocal(rcp[:], ssum[:])
                            nc.vector.tensor_scalar_mul(
                                exp_t[:], exp_t[:], rcp[:])
                            nc.sync.dma_start(
                                attn_o[h, m * 128:(m + 1) * 128, :], exp_t[:])
                            # transpose the normalized tile for ctx
                            for g in range(4):
                                ps_t = psumT.tile([128, 4, 128], F32,
                                                  tag="pt")
                                for t4 in range(4):
                                    nc.tensor.transpose(
                                        ps_t[:, t4],
                                        exp_t[:, (g * 4 + t4) * 128:
                                              (g * 4 + t4 + 1) * 128],
                                        id32[:])
                                dst = at_t[:, g * 4:(g + 1) * 4,
                                           mi * 128:(mi + 1) * 128]
                                if g % 2 == 0:
                                    nc.vector.tensor_copy(dst, ps_t[:])
                                else:
                                    nc.scalar.copy(dst, ps_t[:])
                        # ctx for this 256-col chunk
                        pc = psumC.tile([64, 256], F32, tag="pc")
                        for kt in range(JT):
                            nc.tensor.matmul(
                                pc[:], v_sb[:, kt, h * 64:(h + 1) * 64],
                                at_t[:, kt], start=(kt == 0),
                                stop=(kt == JT - 1))
                        nc.vector.tensor_copy(
                            ctxT_sb[ho:ho + 64, hp,
                                    qc * 256:(qc + 1) * 256], pc[:])

            # ---------------- Phase O: output projection ----------------
            with (
                tc.tile_pool(name="opool", bufs=2) as opool,
                tc.tile_pool(name="psumO", bufs=4, space="PSUM") as psumO,
            ):
                for mt in range(JT):
                    out_t = opool.tile([128, H], F32, tag="out")
                    for n in range(2):
                        po = psumO.tile([128, 512], F32, tag="po")
                        for cc in range(2):
                            nc.tensor.matmul(
                                po[:],
                                ctxT_sb[:, cc, mt * 128:(mt + 1) * 128],
                                wo_sb[:, cc, n * 512:(n + 1) * 512],
                                start=(cc == 0), stop=(cc == 1))
                        nc.scalar.copy(out_t[:, n * 512:(n + 1) * 512], po[:])
                    nc.sync.dma_start(
                        out_o[mt * 128:(mt + 1) * 128, :], out_t[:])

    nc.compile()
    return nc


_NC_CACHE = None


def _get_program():
    global _NC_CACHE
    if _NC_CACHE is None:
        _NC_CACHE = build_program()
    return _NC_CACHE


def _make_in_maps(query, key, value, Wq, Wk, Wv, Wo):
    query = np.asarray(query, dtype=np.float32)
    key = np.asarray(key, dtype=np.float32)
    value = np.asarray(value, dtype=np.float32)
    Wq = np.asarray(Wq, dtype=np.float32)
    Wk = np.asarray(Wk, dtype=np.float32)
    Wv = np.asarray(Wv, dtype=np.float32)
    Wo = np.asarray(Wo, dtype=np.float32)

    # [N,H,D] -> [H, N*D]
    wq_f = np.ascontiguousarray(Wq.transpose(1, 0, 2).reshape(H, N * D))
    wk_f = np.ascontiguousarray(Wk.transpose(1, 0, 2).reshape(H, N * D))
    wv_f = np.ascontiguousarray(Wv.transpose(1, 0, 2).reshape(H, N * D))

    in_maps = []
    for c in range(NC):
        b, g = c // 4, c % 4
        cols = slice(g * NH * D, (g + 1) * NH * D)
        in_maps.append({
            "xq": np.ascontiguousarray(query[b]),
            "xk": np.ascontiguousarray(key[b]),
            "xv": np.ascontiguousarray(value[b]),
            "wq": np.ascontiguousarray(wq_f[:, cols]),
            "wk": np.ascontiguousarray(wk_f[:, cols]),
            "wv": np.ascontiguousarray(wv_f[:, cols]),
            "wo": np.ascontiguousarray(Wo[cols, :]),
        })
    return in_maps


def run(trace=False, **inputs):
    nc = _get_program()
    in_maps = _make_in_maps(**inputs)
    res = bass_utils.run_bass_kernel_spmd(
        nc, in_maps, core_ids=list(range(NC)), trace=trace)
    out = np.zeros((B, S, H), dtype=np.float32)
    attn = np.empty((B, N, S, S), dtype=np.float32)
    for c in range(NC):
        b, g = c // 4, c % 4
        r = res.results[c]
        attn[b, g * NH:(g + 1) * NH] = r["attn_o"]
        out[b] += r["out_o"]
    return (out, attn), res


def kernel(**inputs):
    (out, attn), _ = run(trace=False, **inputs)
    return out, attn
